# revision 18
# baseline (speedup 1.0000x reference)
"""MoE routing kernel (nn_EnhancedBrain) for Trainium2, 8 NeuronCores.

Strategy
--------
The router (mean-pool -> tiny MLP -> softmax -> top-3 -> renormalize) costs
~8 MFLOP vs ~1.7 TFLOP for the expert MLPs, and its only effect on the math
is which 3 of the 8 zone weights are nonzero per batch row.  It runs on host
in float64; the device computes exactly the nonzero-weight expert MLPs.

Sharding: by batch row.  Core c gets rows 2c and 2c+1, each with its 3
selected experts -> 6 equal expert-MLP passes per core, perfectly balanced,
no collectives.  The renormalized top-k weight is folded into that pair's Wb
copy on host.

The PE sustains ~0.5 ns per moving column (P0 power state ~2.0 GHz), so
runtime is set by matmul COUNT x moving length.  BOTH layers run in fp8e4
with perf_mode=DoubleRow, which packs TWO 128-deep contraction subtiles per
matmul - half the matmul count of bf16 at roughly the same per-instruction
cost (6144 MMs x 512 cols/core ~ 1.7 ms vs 9216 for the old fp8/bf16 mix).
Each stationary operand is loaded once and reused by all NTB=4 moving
blocks (4 PSUM banks per accumulation chain-set); rotating the stationary
every 1-2 matmuls measured ~5% slower (LDWEIGHTS not fully hidden).  The
staggered stop positions let ACT/DVE drains overlap the tail matmuls, so
4-bank chain-sets add no PSUM-recycle stalls.  Measured: 273-276 ns/MM
sustained vs ~264 ns pure-streaming ideal; L1-only and full-kernel rates
match, so the residual ~4% is intrinsic to the DR instruction stream.

fp8 scaling (e4m3: max 240, min normal 2^-6 - denormals are the killer):
  - x pre-scaled by 32 (sigma 1 -> 32), Wa pre-scaled by 32 (sigma 1/32 ->
    1); gelu descales via its input scale (gelu(psum/1024)).
  - h = gelu output (range ~[-0.17, 6]) quantizes to fp8 unscaled: values
    below the denormal floor contribute nothing to the output norm.
  - Wb*w pre-scaled by 2048 (sigma w/64 -> 32w); the z accumulator runs in
    the x2048 domain (z := 2048*x, PSUM drains add raw), and a final
    in-place multiply by 1/2048 restores the output scale.
  - The residual copy of x is f32 (DMA'd directly into z), not bf16.
Measured end-to-end rel err of this scheme: ~1.9e-2 (gate 2e-2); the
fp8 L1 alone measures 1.38e-2 (quantization of x/Wa dominates).

Device kernel (per core), transposed [feature, token] layout throughout:
  for each batch row:
    z^T[d, t]  = 2048 * x^T[d, t]                (f32 DMA + DVE in-place mul)
    for each of the 3 experts, over f-chunks of 512:
      h^T[f, t] = fp8(gelu_tanh(Wa^T x^T / 1024))  (PE fp8 DR + ACT,
                                                    PSUM chain over 4 d-pairs,
                                                    written into f-paired tiles)
      z^T      += (2048 w Wb)^T h^T                (PE fp8 DR, PSUM chain over
                                                    f-pairs; DVE add)
    y^T[d, t]  = z^T / 2048                        (DVE in-place mul + DMA)
"""

import numpy as np
import ml_dtypes

import concourse.bass as bass
import concourse.mybir as mybir
import concourse.tile as tile
from concourse import bacc
from concourse.bass_utils import run_bass_kernel_spmd

B, S, D, F = 16, 2048, 1024, 4096
NZONES, TOPK = 8, 3
NCORES = 8
NB = B // NCORES            # batch rows per core = 2
NP = NB * TOPK              # (row, expert) pairs per core = 6
TB = 512                    # matmul moving-dim block (PSUM bank limit)
NTB = S // TB               # 4 moving blocks per row
FC = 512                    # f-chunk produced per L1 step
P = 128
NQ = F // 256               # fp8 DoubleRow f-pair tiles per expert = 16
X_SCALE = 32.0              # host pre-scale on x (fp8 denormal floor)
WA_SCALE = 32.0             # host pre-scale on Wa
WB_SCALE = 2048.0           # host pre-scale on Wb*w

F32 = mybir.dt.float32
FP8 = mybir.dt.float8e4
NPFP8 = ml_dtypes.float8_e4m3
GELU = mybir.ActivationFunctionType.Gelu_apprx_tanh
DR = mybir.MatmulPerfMode.DoubleRow

_compiled_nc = None


def _build_nc(reps=1):
    from contextlib import nullcontext

    nc = bacc.Bacc("TRN2", target_bir_lowering=False)
    xtf = nc.dram_tensor("xtf", [NB, D, S], F32, kind="ExternalInput")
    xt8 = nc.dram_tensor("xt8", [NB, D, S], FP8, kind="ExternalInput")
    wa8 = nc.dram_tensor("wa8", [NP, D, F], FP8, kind="ExternalInput")
    # wb8[pr, q, p, j, :] = fp8(WB_SCALE * w * Wb[e][q*256 + j*128 + p, :])
    wb8 = nc.dram_tensor("wb8", [NP, NQ, P, 2, D], FP8, kind="ExternalInput")
    y = nc.dram_tensor("y", [NB, D, S], F32, kind="ExternalOutput")

    nd, nf, nfc = D // P, FC // P, F // FC
    npair = nd // 2         # 4 fp8 contraction pair-tiles over D
    nqc = FC // 256         # f-pair tiles per chunk = 2

    with tile.TileContext(nc) as tc:
        with (
            tc.tile_pool(name="x8p", bufs=npair + 1) as x8_pool,
            tc.tile_pool(name="zp", bufs=nd) as z_pool,
            tc.tile_pool(name="hp", bufs=7 * nqc) as h_pool,
            tc.tile_pool(name="wap", bufs=2 * npair + 2) as wa_pool,
            tc.tile_pool(name="wbp", bufs=7 * nqc + 2) as wb_pool,
            tc.tile_pool(name="ps1", bufs=4, space="PSUM") as ps1,
            tc.tile_pool(name="ps2", bufs=4, space="PSUM") as ps2,
            tc.For_i(0, reps, 1) if reps > 1 else nullcontext(),
        ):
            for bi in range(NB):
                x8s = []
                for dp in range(npair):
                    x8tile = x8_pool.tile([P, 2, S], FP8, tag="x8")
                    for j in range(2):
                        dc = 2 * dp + j
                        nc.sync.dma_start(
                            x8tile[:, j, :],
                            xt8[bi, dc * P:(dc + 1) * P, :],
                        )
                    x8s.append(x8tile)
                zts = []
                for dc in range(nd):
                    ztile = z_pool.tile([P, S], F32, tag="z")
                    # xtf is pre-scaled by WB_SCALE on host (exact, power of
                    # 2), so the DMA alone initializes the z accumulator.
                    # Activation HWDGE queue: keeps the 16MB/row residual
                    # load from stalling the Wa/x8 stream on the sync queue
                    nc.scalar.dma_start(
                        ztile[:], xtf[bi, dc * P:(dc + 1) * P, :]
                    )
                    zts.append(ztile)

                def emit_l2(chunks):
                    # z^T += (Wb*w*WB_SCALE)^T h^T for a group of f-chunks in
                    # one PSUM chain per (dc, tb); chunks may span expert
                    # boundaries (the accumulation is linear so that is
                    # exact).
                    hts = [t for c in chunks for t in c[0]]
                    wbts = [t for c in chunks for t in c[1]]
                    nch = len(hts)
                    for dc in range(nd):
                        # one PSUM bank per tb block so each Wb stationary
                        # is loaded once and reused by NTB matmuls
                        pzs = [ps2.tile([P, TB], F32, tag="ps2",
                                        name=f"pz{i}")
                               for i in range(NTB)]
                        for qi in range(nch):
                            for i in range(NTB):
                                s = slice(i * TB, (i + 1) * TB)
                                nc.tensor.matmul(
                                    pzs[i][:],
                                    wbts[qi][:, :, dc * P:(dc + 1) * P],
                                    hts[qi][:, :, s],
                                    start=(qi == 0),
                                    stop=(qi == nch - 1),
                                    perf_mode=DR,
                                )
                        for i in range(NTB):
                            s = slice(i * TB, (i + 1) * TB)
                            nc.vector.tensor_tensor(
                                zts[dc][:, s],
                                zts[dc][:, s],
                                pzs[i][:],
                                op=mybir.AluOpType.add,
                            )

                # Software-pipelined: L2 runs in groups of 6 f-chunks with a
                # 1-chunk skew, so the in-order PE covers each gelu tail with
                # later L1 matmuls.  Bigger groups = fewer z read-modify-write
                # drains (measured win: SBUF contention with the PE moving
                # stream); 6 is the largest that fits SBUF alongside the
                # 7-chunk-deep h/wb pools.
                pending = []
                for k in range(TOPK):
                    pr = bi * TOPK + k
                    for fc in range(nfc):
                        f0 = fc * FC
                        wats = []
                        for dp in range(npair):
                            wt = wa_pool.tile([P, 2, FC], FP8, tag="wa")
                            for j in range(2):
                                dc = 2 * dp + j
                                nc.sync.dma_start(
                                    wt[:, j, :],
                                    wa8[pr, dc * P:(dc + 1) * P,
                                        f0:f0 + FC],
                                )
                            wats.append(wt)
                        wbts = []
                        for qc in range(nqc):
                            q = fc * nqc + qc
                            wt = wb_pool.tile([P, 2, D], FP8, tag="wb")
                            nc.scalar.dma_start(wt[:], wb8[pr, q])
                            wbts.append(wt)
                        hts = []
                        for qc in range(nqc):
                            ht = h_pool.tile([P, 2, S], FP8, tag="h")
                            for j in range(2):
                                fi = 2 * qc + j
                                # one PSUM bank per tb block so each Wa
                                # stationary is loaded once per NTB matmuls
                                phs = [ps1.tile([P, TB], F32, tag="ps1",
                                                name=f"ph{i}")
                                       for i in range(NTB)]
                                for dp in range(npair):
                                    for i in range(NTB):
                                        s = slice(i * TB, (i + 1) * TB)
                                        nc.tensor.matmul(
                                            phs[i][:],
                                            wats[dp][:, :,
                                                     fi * P:(fi + 1) * P],
                                            x8s[dp][:, :, s],
                                            start=(dp == 0),
                                            stop=(dp == npair - 1),
                                            perf_mode=DR,
                                        )
                                for i in range(NTB):
                                    s = slice(i * TB, (i + 1) * TB)
                                    nc.scalar.activation(
                                        ht[:, j, s], phs[i][:], GELU,
                                        scale=1.0 / (X_SCALE * WA_SCALE),
                                    )
                            hts.append(ht)
                        pending.append((hts, wbts))
                        if len(pending) == 7:
                            emit_l2(pending[:6])
                            pending = pending[6:]
                emit_l2(pending)
                for dc in range(nd):
                    nc.vector.tensor_scalar_mul(
                        zts[dc][:], zts[dc][:], 1.0 / WB_SCALE
                    )
                    nc.gpsimd.dma_start(
                        y[bi, dc * P:(dc + 1) * P, :], zts[dc][:]
                    )
    nc.compile()
    return nc


def _route(x, W1, b1, W2, b2):
    """Host router in float64; reproduces jax.lax.top_k tie-breaking."""
    pooled = x.mean(axis=1, dtype=np.float64)
    h = np.tanh(pooled @ W1.astype(np.float64) + b1.astype(np.float64))
    logits = h @ W2.astype(np.float64) + b2.astype(np.float64)
    e = np.exp(logits - logits.max(axis=-1, keepdims=True))
    probs = e / e.sum(axis=-1, keepdims=True)
    top_i = np.argsort(-probs, axis=-1, kind="stable")[:, :TOPK]
    top_p = np.take_along_axis(probs, top_i, axis=-1)
    top_w = top_p / top_p.sum(axis=-1, keepdims=True)
    return top_i, top_w


def make_in_maps(x, W1, b1, W2, b2, Wa, Wb):
    x = np.ascontiguousarray(np.asarray(x, dtype=np.float32))
    Wa = np.asarray(Wa, dtype=np.float32)
    Wb = np.asarray(Wb, dtype=np.float32)

    top_i, top_w = _route(
        x, np.asarray(W1), np.asarray(b1), np.asarray(W2), np.asarray(b2)
    )

    Wa_f8 = (Wa * np.float32(WA_SCALE)).astype(NPFP8)
    in_maps = []
    for c in range(NCORES):
        rows = [NB * c + i for i in range(NB)]
        xT = x[rows].transpose(0, 2, 1)
        xtfc = np.ascontiguousarray(xT * np.float32(WB_SCALE))
        xtc8 = np.ascontiguousarray((xT * np.float32(X_SCALE)).astype(NPFP8))
        wa_l, wb_l = [], []
        for i, b in enumerate(rows):
            for k in range(TOPK):
                e = int(top_i[b, k])
                wa_l.append(Wa_f8[e])
                wbs = np.clip(
                    Wb[e] * np.float32(top_w[b, k] * WB_SCALE), -240.0, 240.0
                ).astype(NPFP8)
                # [F, D] -> [NQ, 128, 2, D] with f = q*256 + j*128 + p
                wb_l.append(
                    wbs.reshape(NQ, 2, P, D).transpose(0, 2, 1, 3)
                )
        in_maps.append({
            "xtf": xtfc,
            "xt8": xtc8,
            "wa8": np.ascontiguousarray(np.stack(wa_l)),
            "wb8": np.ascontiguousarray(np.stack(wb_l)),
        })
    return in_maps


def kernel(x, W1, b1, W2, b2, Wa, Wb):
    global _compiled_nc
    if _compiled_nc is None:
        _compiled_nc = _build_nc()
    nc = _compiled_nc

    in_maps = make_in_maps(x, W1, b1, W2, b2, Wa, Wb)
    res = run_bass_kernel_spmd(nc, in_maps, core_ids=list(range(NCORES)))

    y = np.empty((B, S, D), dtype=np.float32)
    for c in range(NCORES):
        yt = res.results[c]["y"]                      # [NB, D, S]
        for i in range(NB):
            y[NB * c + i] = yt[i].T
    return y
